# revision 1
# baseline (speedup 1.0000x reference)
"""3-layer GCN (B=32 graphs, N=512 nodes, D=512 feats) on 8 trn2 NeuronCores.

Sharding: data-parallel over graphs — 4 graphs per core, weights replicated.

Math per graph g, per layer l:  h <- adj @ (h @ Wl) + bl  (relu on l=0,1).

Device layout trick: each layer is two matmuls whose contraction dims
alternate (features d, then nodes m).  We chain them with no on-device
transposes by keeping the layer input as G = H^T (feature-on-partition):
  MM1: S[n_i, e]   = sum_d G[d, n_i]^T W[d, e]     (lhsT=G chunk, rhs=W)
  MM2: G'[e_j, n]  = sum_m S[m, e_j]^T A^T[m, n]   (lhsT=S chunk, rhs=A^T)
MM2's output is already H'^T, feeding the next layer's MM1.  The host
pre-transposes batch_graph (-> X^T) and adj (-> A^T) and transposes the
final output back; those are free w.r.t. HW kernel time.
"""

import numpy as np

import concourse.bass as bass
import concourse.mybir as mybir
import concourse.tile as tile
from concourse import bacc
from concourse.bass_utils import run_bass_kernel_spmd

B, N, D = 32, 512, 512
N_CORES = 8
GPC = B // N_CORES  # graphs per core
P = 128
KO = D // P  # 128-partition chunks per 512 dim

# Matmul input dtype: float32 (exact, 4 cyc/row), float32r (fast fp32 path,
# 1 cyc/row at N>=256), bfloat16 (1 cyc/row, casts inputs).
MM_DT = mybir.dt.float32r

_CACHE = {}
LAST_RESULTS = None


def _build(reps=1, order="bfs"):
    f32 = mybir.dt.float32
    nc = bacc.Bacc("TRN2", target_bir_lowering=False, debug=False)

    xt = nc.dram_tensor("xt", [GPC, D, N], MM_DT, kind="ExternalInput").ap()
    at = nc.dram_tensor("at", [GPC, N, N], MM_DT, kind="ExternalInput").ap()
    w_dram = [
        nc.dram_tensor(f"w{l}", [D, D], MM_DT, kind="ExternalInput").ap()
        for l in range(3)
    ]
    b_dram = [
        nc.dram_tensor(f"b{l}", [D], f32, kind="ExternalInput").ap() for l in range(3)
    ]
    out = nc.dram_tensor("out", [GPC, D, N], f32, kind="ExternalOutput").ap()

    relu = mybir.ActivationFunctionType.Relu

    from contextlib import ExitStack

    with tile.TileContext(nc) as tc:
        with (
            tc.tile_pool(name="weights", bufs=1) as wpool,
            tc.tile_pool(name="gbuf", bufs=8) as gpool,
            tc.tile_pool(name="adj", bufs=4) as apool,
            tc.tile_pool(name="sbuf_s", bufs=4) as spool,
            tc.tile_pool(name="outp", bufs=3) as opool,
            tc.tile_pool(name="psum", bufs=4, space="PSUM") as pspool,
            ExitStack() as loop_ctx,
        ):
            # Weight/bias loads: layer 0 first (gates the first matmuls), the
            # rest after the graph inputs so they don't compete for DMA early.
            w_sb = [
                wpool.tile([P, KO, D], MM_DT, tag=f"w{l}", name=f"w_sb{l}")
                for l in range(3)
            ]
            b_sb = [
                wpool.tile([P, KO], f32, tag=f"b{l}", name=f"b_sb{l}")
                for l in range(3)
            ]

            def load_weights(l):
                wr = w_dram[l].rearrange("(ko p) e -> p ko e", p=P)
                for k in range(KO):
                    nc.sync.dma_start(w_sb[l][:, k, :], wr[:, k, :])
                nc.sync.dma_start(
                    b_sb[l][:], b_dram[l].rearrange("(ko p) -> p ko", p=P)
                )

            load_weights(0)

            if reps > 1:
                loop_ctx.enter_context(tc.For_i(0, reps, 1))

            gts, ats = [], []
            for g in range(GPC):
                gt = gpool.tile([P, KO, N], MM_DT, tag="g")
                gr = xt[g].rearrange("(ko p) n -> p ko n", p=P)
                for k in range(KO):
                    nc.sync.dma_start(gt[:, k, :], gr[:, k, :])
                a_t = apool.tile([P, KO, N], MM_DT, tag="a")
                ar = at[g].rearrange("(ko p) n -> p ko n", p=P)
                for k in range(KO):
                    nc.sync.dma_start(a_t[:, k, :], ar[:, k, :])
                gts.append(gt)
                ats.append(a_t)
                if g == 0 and reps == 1:
                    load_weights(1)
                    load_weights(2)
            if reps > 1:
                load_weights(1)
                load_weights(2)

            if order == "bfs":
                lg_order = [(l, g) for l in range(3) for g in range(GPC)]
            else:
                lg_order = [(l, g) for g in range(GPC) for l in range(3)]
            for l, g in lg_order:
                last = l == 2
                gt, a_t = gts[g], ats[g]
                # MM1: S[n_i, :] = sum_k G_k[:, n_i].T @ W_k  (node-on-part.)
                s_t = spool.tile([P, KO, D], MM_DT, tag="s")
                for i in range(KO):
                    ps = pspool.tile([P, D], f32, tag="ps")
                    for k in range(KO):
                        nc.tensor.matmul(
                            ps[:],
                            lhsT=gt[:, k, P * i : P * (i + 1)],
                            rhs=w_sb[l][:, k, :],
                            start=(k == 0),
                            stop=(k == KO - 1),
                        )
                    nc.vector.tensor_copy(s_t[:, i, :], ps[:])

                # MM2: G'[e_j, :] = sum_k S_k[:, e_j].T @ A^T_k  (feat-on-p.)
                pool = opool if last else gpool
                g_next = pool.tile(
                    [P, KO, N], f32 if last else MM_DT, tag=("o" if last else "g")
                )
                for j in range(KO):
                    pz = pspool.tile([P, N], f32, tag="pz")
                    for k in range(KO):
                        nc.tensor.matmul(
                            pz[:],
                            lhsT=s_t[:, k, P * j : P * (j + 1)],
                            rhs=a_t[:, k, :],
                            start=(k == 0),
                            stop=(k == KO - 1),
                        )
                    if last:
                        nc.vector.tensor_scalar_add(
                            g_next[:, j, :], pz[:], b_sb[l][:, j : j + 1]
                        )
                        nc.sync.dma_start(
                            out[g].rearrange("(ko p) n -> p ko n", p=P)[:, j, :],
                            g_next[:, j, :],
                        )
                    else:
                        nc.scalar.activation(
                            g_next[:, j, :],
                            pz[:],
                            relu,
                            bias=b_sb[l][:, j : j + 1],
                        )
                gts[g] = g_next

    nc.compile()
    return nc


def _round_f32r(x):
    """Round fp32 -> fp32r (TF32-like E8M11) on host: RNE at mantissa bit 12.

    The device fp32r memory format is an fp32 word with the low 12 mantissa
    bits zero, so pre-rounding lets the kernel DMA inputs with no cast.
    """
    b = np.ascontiguousarray(x, np.float32).view(np.uint32)
    bias = np.uint32(0x7FF) + ((b >> np.uint32(12)) & np.uint32(1))
    b = (b + bias) & np.uint32(0xFFFFF000)
    return b.view(np.float32)


def kernel(batch_graph, adj, W0, b0, W1, b1, W2, b2, trace=False):
    global LAST_RESULTS
    if "nc" not in _CACHE:
        _CACHE["nc"] = _build()
    nc = _CACHE["nc"]

    xt = _round_f32r(np.asarray(batch_graph, np.float32).transpose(0, 2, 1))
    at = _round_f32r(np.asarray(adj, np.float32).transpose(0, 2, 1))
    ws = [_round_f32r(np.asarray(w, np.float32)) for w in (W0, W1, W2)]
    bs = [np.ascontiguousarray(np.asarray(b, np.float32)) for b in (b0, b1, b2)]

    in_maps = []
    for c in range(N_CORES):
        sl = slice(c * GPC, (c + 1) * GPC)
        in_maps.append(
            {
                "xt": np.ascontiguousarray(xt[sl]),
                "at": np.ascontiguousarray(at[sl]),
                "w0": ws[0], "b0": bs[0],
                "w1": ws[1], "b1": bs[1],
                "w2": ws[2], "b2": bs[2],
            }
        )

    try:
        res = run_bass_kernel_spmd(
            nc, in_maps, core_ids=list(range(N_CORES)), trace=trace
        )
    except ModuleNotFoundError:
        # Tracing was requested (arg or BASS_TRACE env) but this environment
        # lacks the axon NTFF profile hook; rerun without the trace path.
        import os

        os.environ["BASS_NEVER_TRACE"] = "1"
        try:
            res = run_bass_kernel_spmd(
                nc, in_maps, core_ids=list(range(N_CORES)), trace=False
            )
        finally:
            del os.environ["BASS_NEVER_TRACE"]
    LAST_RESULTS = res
    outs = [r["out"].transpose(0, 2, 1) for r in res.results]  # [GPC, N, D] each
    return np.ascontiguousarray(np.concatenate(outs, axis=0), dtype=np.float32)

